# revision 1
# baseline (speedup 1.0000x reference)
"""Trainium2 Bass kernel for nn_CustomLayerMKM: y = x @ kron(W2, W1).T + bias.

x: (8, 8192, 1024) fp32, W1/W2: (32, 32), bias: (1024,).
Data-parallel over the 65536 tokens across 8 NeuronCores; weights replicated.

On-device algorithm (default, _build_bd): per 128-token chunk, reshape each
token to X (32x32) and compute Y = W2 @ X @ W1.T using block-diagonal
128x128 matmuls (lhsT = kron(I4, W.T), float32r at 1 cyc/row) between PE
transpose stages that move data token-major <-> feature-major. A dense-K
fallback (_build) materializes kron(W2, W1) on host.
"""

import functools
import numpy as np

B, S, IN, OUT = 8, 8192, 1024, 1024
N_CORES = 8
TOKENS = B * S
TOK_PER_CORE = TOKENS // N_CORES  # 8192
SUP = 512  # tokens per superblock


@functools.lru_cache(maxsize=4)
def _build(n_tok=TOK_PER_CORE, use_f32r=True, reps=1):
    import concourse.bass as bass  # noqa: F401
    import concourse.tile as tile
    from concourse import bacc, mybir
    from concourse.masks import make_identity
    from contextlib import ExitStack

    f32 = mybir.dt.float32
    mmdt = mybir.dt.float32r if use_f32r else f32

    assert n_tok % SUP == 0
    nc = bacc.Bacc("TRN2", target_bir_lowering=False, debug=False,
                   num_devices=N_CORES)
    x = nc.dram_tensor("x", [n_tok, IN], f32, kind="ExternalInput").ap()
    # kt[p, (kb*8+m)*128 + i] = K.T[kb*128+p, m*128+i]  (host-prepared)
    kt = nc.dram_tensor("kt", [128, 8192], mmdt, kind="ExternalInput").ap()
    bb = nc.dram_tensor("bias_bcast", [128, OUT], f32, kind="ExternalInput").ap()
    y = nc.dram_tensor("y", [n_tok, OUT], f32, kind="ExternalOutput").ap()

    with tile.TileContext(nc) as tc, ExitStack() as ctx:
        const = ctx.enter_context(tc.tile_pool(name="const", bufs=1))
        xpool = ctx.enter_context(tc.tile_pool(name="xin", bufs=2))
        xtpool = ctx.enter_context(tc.tile_pool(name="xt", bufs=2))
        ypool = ctx.enter_context(tc.tile_pool(name="ysb", bufs=2))
        ytpool = ctx.enter_context(tc.tile_pool(name="ytok", bufs=2))
        ps_in = ctx.enter_context(tc.tile_pool(name="ps_in", bufs=2, space="PSUM"))
        ps_mm = ctx.enter_context(tc.tile_pool(name="ps_mm", bufs=2, space="PSUM"))
        ps_out = ctx.enter_context(tc.tile_pool(name="ps_out", bufs=2, space="PSUM"))

        ident = const.tile([128, 128], f32)
        make_identity(nc, ident[:])
        ktile = const.tile([128, 8192], mmdt)
        nc.sync.dma_start(ktile[:], kt[:, :])
        btile = const.tile([128, OUT], f32)
        nc.sync.dma_start(btile[:], bb[:, :])

        def body():
            for sb in range(n_tok // SUP):
                one_superblock(sb)

        def one_superblock(sb):
            r0 = sb * SUP
            # ---- load 512 tokens: SBUF [p=tok%128, free=(a, f)] ----
            xin = xpool.tile([128, 4 * IN], f32)
            nc.sync.dma_start(
                xin[:].rearrange("p (a f) -> p a f", a=4),
                x[r0:r0 + SUP, :].rearrange("(a p) f -> p a f", p=128))
            # ---- T-in: feature-major XT [p = f%128, free=(kb, a, tq)] ----
            xt_sb = xtpool.tile([128, 4096], mmdt)
            for fb in range(8):
                pin = ps_in.tile([128, 512], f32)
                for a in range(4):
                    nc.tensor.transpose(
                        pin[:, a * 128:(a + 1) * 128],
                        xin[:, a * IN + fb * 128: a * IN + (fb + 1) * 128],
                        ident[:])
                nc.scalar.copy(xt_sb[:, fb * 512:(fb + 1) * 512], pin[:])
            # ---- dense matmul: y_sb [p=i%128, free=(m, a, tq)] ----
            y_sb = ypool.tile([128, 4096], f32)
            for m in range(8):
                pm = ps_mm.tile([128, 512], f32)
                for kb in range(8):
                    nc.tensor.matmul(
                        pm[:],
                        ktile[:, (kb * 8 + m) * 128:(kb * 8 + m + 1) * 128],
                        xt_sb[:, kb * 512:(kb + 1) * 512],
                        start=(kb == 0), stop=(kb == 7))
                nc.scalar.copy(y_sb[:, m * 512:(m + 1) * 512], pm[:])
            # ---- T-out + bias: ytok [p=tok%128, free=(a, i)] ----
            yt = ytpool.tile([128, 4 * OUT], f32)
            for a in range(4):
                pot = ps_out.tile([128, 1024], f32)
                for m in range(8):
                    nc.tensor.transpose(
                        pot[:, m * 128:(m + 1) * 128],
                        y_sb[:, m * 512 + a * 128: m * 512 + (a + 1) * 128],
                        ident[:])
                nc.vector.tensor_add(
                    yt[:, a * OUT:(a + 1) * OUT], pot[:], btile[:])
            nc.sync.dma_start(
                y[r0:r0 + SUP, :].rearrange("(a p) f -> p a f", p=128),
                yt[:].rearrange("p (a f) -> p a f", a=4))

        if reps == 1:
            body()
        else:
            with tc.For_i(0, reps, 1):
                body()

    nc.compile()
    return nc


@functools.lru_cache(maxsize=6)
def _build_bd(n_tok=TOK_PER_CORE, mode="f32r", reps=1):
    """Block-diagonal factored kernel: MM stages are full 128x128 matmuls with
    lhsT = kron(I4, W.T), processing 4 consecutive j2 (resp. i1) per call.
    Unlike tile_position col-tiling this is f32r-eligible (1 cyc/row at N>=256).

    mode: "f32r" (x/z rounded to f32r at the two MM inputs, rest fp32),
          "f32" (exact), "bf16" (everything bf16 on chip).
    """
    import concourse.bass as bass  # noqa: F401
    import concourse.tile as tile
    from concourse import bacc, mybir
    from concourse.masks import make_identity
    from contextlib import ExitStack

    f32 = mybir.dt.float32
    mmdt = {"f32": f32, "f32r": mybir.dt.float32r,
            "bf16": mybir.dt.bfloat16}[mode]
    flowdt = mybir.dt.bfloat16 if mode == "bf16" else f32

    assert n_tok % SUP == 0
    nc = bacc.Bacc("TRN2", target_bir_lowering=False, debug=False,
                   num_devices=N_CORES)
    x = nc.dram_tensor("x", [n_tok, IN], f32, kind="ExternalInput").ap()
    w1bd = nc.dram_tensor("w1bd", [128, 128], mmdt, kind="ExternalInput").ap()
    w2bd = nc.dram_tensor("w2bd", [128, 128], mmdt, kind="ExternalInput").ap()
    bb = nc.dram_tensor("bias_bcast", [128, OUT], f32, kind="ExternalInput").ap()
    y = nc.dram_tensor("y", [n_tok, OUT], f32, kind="ExternalOutput").ap()

    with tile.TileContext(nc) as tc, ExitStack() as ctx:
        const = ctx.enter_context(tc.tile_pool(name="const", bufs=1))
        xpool = ctx.enter_context(tc.tile_pool(name="xin", bufs=2))
        xtpool = ctx.enter_context(tc.tile_pool(name="xt", bufs=2))
        ztokp = ctx.enter_context(tc.tile_pool(name="ztok", bufs=2))
        ztsbp = ctx.enter_context(tc.tile_pool(name="ztsb", bufs=1))
        ytokp = ctx.enter_context(tc.tile_pool(name="ytok", bufs=2))
        ps_tA = ctx.enter_context(tc.tile_pool(name="ps_tA", bufs=3, space="PSUM"))
        ps_tB = ctx.enter_context(tc.tile_pool(name="ps_tB", bufs=5, space="PSUM"))

        ident = const.tile([128, 128], f32)
        make_identity(nc, ident[:])
        identf = const.tile([128, 128], flowdt)
        make_identity(nc, identf[:])
        w1tt = const.tile([128, 128], mmdt)
        nc.sync.dma_start(w1tt[:], w1bd[:, :])
        w2tt = const.tile([128, 128], mmdt)
        nc.sync.dma_start(w2tt[:], w2bd[:, :])
        btile = const.tile([128, OUT], f32)
        nc.sync.dma_start(btile[:], bb[:, :])

        def one_superblock(sb):
            r0 = sb * SUP
            xin = xpool.tile([128, 4 * IN], f32)
            nc.sync.dma_start(
                xin[:].rearrange("p (a f) -> p a f", a=4),
                x[r0:r0 + SUP, :].rearrange("(a p) f -> p a f", p=128))
            # ---- T-in: XT [p=(b,j1), free=(g, a, tq)], dtype mmdt ----
            xt_sb = xtpool.tile([128, 4096], mmdt)
            for g in range(8):
                pin = ps_tA.tile([128, 512], f32, name="pin", tag="tA")
                for a in range(4):
                    nc.tensor.transpose(
                        pin[:, a * 128:(a + 1) * 128],
                        xin[:, a * IN + g * 128: a * IN + (g + 1) * 128],
                        ident[:])
                nc.scalar.copy(xt_sb[:, g * 512:(g + 1) * 512], pin[:])

            yt = ytokp.tile([128, 4 * OUT], f32)
            # ---- fused MM1+T-mid: one matmul per (g,k): lhsT = XT-slice
            # (stationary), rhs = w1bd -> out = Z.T block [t, (b,i1)];
            # zf = i1*32 + j2, j2 = 4*g+b = 16*p0+4*gg+b ----
            zt_k = [ztokp.tile([128, 1024], flowdt, name=f"ztk{k}",
                               tag=f"ztok{k}")
                    for k in range(4)]
            for p0 in range(2):
                for k in range(4):
                    tm = ps_tB.tile([128, 512], f32, name="tm", tag="tB")
                    for gg in range(4):
                        g = 4 * p0 + gg
                        nc.tensor.matmul(
                            tm[:, gg * 128:(gg + 1) * 128],
                            xt_sb[:, g * 512 + k * 128:
                                  g * 512 + k * 128 + 128],
                            w1tt[:],
                            start=True, stop=True)
                    dest = zt_k[k][:].rearrange(
                        "p (i1 po gg b) -> p po gg b i1",
                        i1=32, po=2, gg=4, b=4)[:, p0:p0 + 1]
                    src = tm[:].rearrange(
                        "p (u gg b i1) -> p u gg b i1", u=1, gg=4, b=4, i1=32)
                    nc.vector.tensor_copy(dest, src)
            # ---- T-in2: ZT [p=(d,j2), (h, k, t)], dtype mmdt ----
            zt_sb = ztsbp.tile([128, 4096], mmdt)
            for k in range(4):
                for hp in range(2):
                    ti2 = ps_tA.tile([128, 512], flowdt, name="ti2", tag="tA")
                    for hh in range(4):
                        h = 4 * hp + hh
                        nc.tensor.transpose(
                            ti2[:, hh * 128:(hh + 1) * 128],
                            zt_k[k][:, h * 128:(h + 1) * 128],
                            identf[:])
                    dest = zt_sb[:].rearrange(
                        "p (h k t) -> p h k t", h=8, k=4, t=128
                    )[:, 4 * hp:4 * hp + 4, k:k + 1]
                    src = ti2[:].rearrange(
                        "p (h u t) -> p h u t", h=4, u=1, t=128)
                    nc.scalar.copy(dest, src)
            # ---- fused MM2+T-out: lhsT = ZT-slice, rhs = w2bd ->
            # out = Y.T block [t, (d,i2)]; yf = i2*32+16*q0+4*hh+d ----
            for q0 in range(2):
                for k in range(4):
                    to = ps_tB.tile([128, 512], f32, name="to", tag="tB")
                    for hh in range(4):
                        h = 4 * q0 + hh
                        nc.tensor.matmul(
                            to[:, hh * 128:(hh + 1) * 128],
                            zt_sb[:, h * 512 + k * 128:
                                  h * 512 + k * 128 + 128],
                            w2tt[:],
                            start=True, stop=True)
                    dest = yt[:, k * OUT:(k + 1) * OUT].rearrange(
                        "p (i2 q hh d) -> p q hh d i2",
                        i2=32, q=2, hh=4, d=4)[:, q0:q0 + 1]
                    bsrc = btile[:].rearrange(
                        "p (i2 q hh d) -> p q hh d i2",
                        i2=32, q=2, hh=4, d=4)[:, q0:q0 + 1]
                    src = to[:].rearrange(
                        "p (u hh d i2) -> p u hh d i2",
                        u=1, hh=4, d=4, i2=32)
                    nc.vector.tensor_add(dest, src, bsrc)
            nc.sync.dma_start(
                y[r0:r0 + SUP, :].rearrange("(a p) f -> p a f", p=128),
                yt[:].rearrange("p (a f) -> p a f", a=4))

        def body():
            for sb in range(n_tok // SUP):
                one_superblock(sb)

        if reps == 1:
            body()
        else:
            with tc.For_i(0, reps, 1):
                body()

    nc.compile()
    return nc


@functools.lru_cache(maxsize=6)
def _build_v2(n_tok=TOK_PER_CORE, sup=512, reps=1):
    """v2: bf16 HBM I/O, host-side pre-transposed feature-major input.

    Per superblock of `sup` tokens (sup=512):
      MM1 (data-stationary): lhsT = XT tile g slice [p=(j2lo,j1), c=tok],
        rhs = w1k = kron(I4, W1.T) -> PSUM token-major Z block [t, (b,i1)].
      copy-1 (ACT): PSUM -> SBUF z' bf16, layout [p=t, (q, g, b, i1)].
      T2 (PE transpose): in_ AP gathers cols (d, jh, jl) = feature
        (i1=4m+d, j2=4jh+jl) -> PSUM [p=(d,j2), t] feat-major, permuted.
      copy-2 (GpSimd): PSUM -> SBUF zsb bf16 [p=(d,j2), (m, q, t)].
      MM2 (data-stationary): lhsT = zsb slice, rhs = w2c (cols (i2,d)) ->
        PSUM token-major Y block [t, (i2, d)].
      copy-3 (DVE): tensor_add with bias -> yt bf16 [p=t, (q, f_out)],
        dest AP scatters (i2, d) -> f_out = 32*i2 + 4m + d.
      DMA store token-major bf16.
    """
    import concourse.bass as bass  # noqa: F401
    import concourse.tile as tile
    from concourse import bacc, mybir
    from concourse.masks import make_identity
    from contextlib import ExitStack

    f32 = mybir.dt.float32
    bf16 = mybir.dt.bfloat16

    assert n_tok % sup == 0 and sup % 128 == 0
    nq = sup // 128
    nc = bacc.Bacc("TRN2", target_bir_lowering=False, debug=False,
                   num_devices=N_CORES)
    # xt[p, g*n_tok + t] = x[t, 128g + p]
    xt = nc.dram_tensor("xt", [128, 8 * n_tok], bf16, kind="ExternalInput").ap()
    w1k = nc.dram_tensor("w1k", [128, 128], bf16, kind="ExternalInput").ap()
    w2c = nc.dram_tensor("w2c", [128, 128], bf16, kind="ExternalInput").ap()
    y = nc.dram_tensor("y", [n_tok, OUT], bf16, kind="ExternalOutput").ap()

    with tile.TileContext(nc) as tc, ExitStack() as ctx:
        const = ctx.enter_context(tc.tile_pool(name="const", bufs=1))
        xpool = ctx.enter_context(tc.tile_pool(name="xin", bufs=2))
        zpool = ctx.enter_context(tc.tile_pool(name="zp", bufs=2))
        zsbp = ctx.enter_context(tc.tile_pool(name="zsb", bufs=2))
        ypool = ctx.enter_context(tc.tile_pool(name="yt", bufs=2))
        ps1 = ctx.enter_context(tc.tile_pool(name="ps1", bufs=2, space="PSUM"))
        ps2 = ctx.enter_context(tc.tile_pool(name="ps2", bufs=2, space="PSUM"))
        ps3 = ctx.enter_context(tc.tile_pool(name="ps3", bufs=2, space="PSUM"))

        identf = const.tile([128, 128], bf16)
        make_identity(nc, identf[:])
        w1t = const.tile([128, 128], bf16)
        nc.sync.dma_start(w1t[:], w1k[:, :])
        w2t = const.tile([128, 128], bf16)
        nc.sync.dma_start(w2t[:], w2c[:, :])

        def one_superblock(sb):
            r0 = sb * sup
            xin = xpool.tile([128, 8 * sup], bf16)
            nc.sync.dma_start(
                xin[:], xt[:, sb * 8 * sup:(sb + 1) * 8 * sup])
            # ---- MM1: token-major Z blocks ----
            # z' col = q*1024 + mm*128 + 4*j2 + d  (j2 = 4*g + b)
            # MM1 out col (w1k col order) = 16*mm + 4*b + d
            zt = zpool.tile([128, nq * 1024], bf16)
            zt_w = zt[:].rearrange(
                "p (q mm jh bd) -> p q mm jh bd", q=nq, mm=8, jh=8)
            for g in range(8):
                pg = ps1.tile([128, sup], f32, name="pg", tag="ps1")
                for q in range(nq):
                    nc.tensor.matmul(
                        pg[:, q * 128:(q + 1) * 128],
                        xin[:, g * sup + q * 128: g * sup + (q + 1) * 128],
                        w1t[:], start=True, stop=True)
                nc.scalar.copy(
                    zt_w[:, :, :, g],
                    pg[:].rearrange("p (q mm bd) -> p q mm bd", q=nq, mm=8))
            # ---- per m: T2 -> copy2 (DVE 2x) -> MM2 -> copy3 (split) ----
            yt = ypool.tile([128, nq * OUT], bf16)
            yt_w = yt[:].rearrange(
                "p (q i2 mm d) -> p q mm i2 d", q=nq, i2=32, mm=8, d=4)
            for m in range(8):
                pt = ps2.tile([128, sup], bf16, name="pt", tag="ps2")
                for q in range(nq):
                    nc.tensor.transpose(
                        pt[:, q * 128:(q + 1) * 128],
                        zt[:, q * 1024 + m * 128: q * 1024 + (m + 1) * 128],
                        identf[:])
                zm = zsbp.tile([128, sup], bf16, name=f"zm{m}", tag=f"zm{m}")
                nc.vector.tensor_copy(zm[:], pt[:])
                py = ps3.tile([128, sup], f32, name="py", tag="ps3")
                for q in range(nq):
                    nc.tensor.matmul(
                        py[:, q * 128:(q + 1) * 128],
                        zm[:, q * 128:(q + 1) * 128],
                        w2t[:], start=True, stop=True)
                src = py[:].rearrange("p (q i2 d) -> p q i2 d", q=nq, i2=32)
                if m >= 3:
                    nc.vector.tensor_copy(yt_w[:, :, m], src)
                else:
                    nc.scalar.copy(yt_w[:, :, m], src)
            nc.sync.dma_start(
                y[r0:r0 + sup, :].rearrange("(q p) f -> p q f", p=128),
                yt[:].rearrange("p (q f) -> p q f", q=nq))

        def body():
            for sb in range(n_tok // sup):
                one_superblock(sb)

        if reps == 1:
            body()
        else:
            with tc.For_i(0, reps, 1):
                body()

    nc.compile()
    return nc


def _prep_v2(x_core_f32, weight_1, weight_2, bias, sup=512):
    """Host-side prep for one core: feature-major bf16 x, superblock-major
    so each superblock's slab is contiguous per partition:
    xt[p, sb*(8*sup) + g*sup + t] = x[sb*sup + t, 128g + p]."""
    import ml_dtypes
    bf = ml_dtypes.bfloat16
    n_tok = x_core_f32.shape[0]
    nsb = n_tok // sup
    xt = np.ascontiguousarray(
        x_core_f32.astype(bf).reshape(nsb, sup, 8, 128).transpose(3, 0, 2, 1)
    ).reshape(128, 8 * n_tok)
    return xt


def _prep_v2_weights(weight_1, weight_2, bias, sup=512):
    import ml_dtypes
    bf = ml_dtypes.bfloat16
    nq = sup // 128
    w1 = np.asarray(weight_1, dtype=np.float32)
    w2 = np.asarray(weight_2, dtype=np.float32)
    b = np.asarray(bias, dtype=np.float32)
    # w1k[32b + j1, 16*mm + 4*b + d] = W1[4*mm + d, j1]
    w1k = np.zeros((128, 128), dtype=np.float32)
    b_idx, j1_idx, i1_idx = np.meshgrid(
        np.arange(4), np.arange(32), np.arange(32), indexing="ij")
    mm_idx, d_idx = i1_idx // 4, i1_idx % 4
    w1k[32 * b_idx + j1_idx, 16 * mm_idx + 4 * b_idx + d_idx] = \
        w1.T[np.zeros_like(b_idx) + j1_idx, i1_idx]
    w1k = w1k.astype(bf)
    # w2c[4*j2 + d, 4*i2 + d] = W2[i2, j2]
    w2c = np.zeros((128, 128), dtype=np.float32)
    d_idx, j2_idx, i2_idx = np.meshgrid(
        np.arange(4), np.arange(32), np.arange(32), indexing="ij")
    w2c[4 * j2_idx + d_idx, 4 * i2_idx + d_idx] = np.broadcast_to(
        w2.T[None, :, :], (4, 32, 32))
    w2c = w2c.astype(bf)
    return w1k, w2c


def _prep_weights_bd(weight_1, weight_2, bias, mode):
    import ml_dtypes
    w1 = np.asarray(weight_1, dtype=np.float32)
    w2 = np.asarray(weight_2, dtype=np.float32)
    b = np.asarray(bias, dtype=np.float32)
    wdt = ml_dtypes.bfloat16 if mode == "bf16" else np.float32
    eye4 = np.eye(4, dtype=np.float32)
    w1bd = np.ascontiguousarray(np.kron(eye4, w1.T).astype(wdt))
    w2bd = np.ascontiguousarray(np.kron(eye4, w2.T).astype(wdt))
    bias_bcast = np.ascontiguousarray(np.broadcast_to(b, (128, OUT)))
    return w1bd, w2bd, bias_bcast


def _prep_weights(weight_1, weight_2, bias):
    w1 = np.asarray(weight_1, dtype=np.float32)
    w2 = np.asarray(weight_2, dtype=np.float32)
    b = np.asarray(bias, dtype=np.float32)
    K = np.kron(w2, w1)  # (OUT, IN)
    KT = np.ascontiguousarray(K.T)  # (IN, OUT); lhsT[f, i] = K[i, f]
    # kt_host[p, (kb*8+m)*128+i] = KT[kb*128+p, m*128+i]
    kt_host = np.ascontiguousarray(
        KT.reshape(8, 128, 8, 128).transpose(1, 0, 2, 3).reshape(128, 8 * 1024))
    bias_bcast = np.ascontiguousarray(np.broadcast_to(b, (128, OUT)))
    return kt_host, bias_bcast


LAST_RESULT = None


V2_SUP = 512


def build_for_mode(n_tok, mode, reps):
    if mode == "v2":
        return _build_v2(n_tok, V2_SUP, reps)
    if mode in ("dense", "dense_f32r"):
        return _build(n_tok, mode == "dense_f32r", reps)
    assert mode.startswith("bd_"), mode
    return _build_bd(n_tok, mode[3:], reps)


def prep_in_maps(x, weight_1, weight_2, bias, n_tok, mode):
    xf = np.ascontiguousarray(np.asarray(x, dtype=np.float32).reshape(-1, IN))
    assert xf.shape[0] == n_tok * N_CORES, (xf.shape, n_tok)
    if mode == "v2":
        w1k, w2c = _prep_v2_weights(weight_1, weight_2, bias, V2_SUP)
        return [
            {"xt": _prep_v2(xf[i * n_tok:(i + 1) * n_tok],
                            weight_1, weight_2, bias, V2_SUP),
             "w1k": w1k, "w2c": w2c}
            for i in range(N_CORES)
        ]
    if mode in ("dense", "dense_f32r"):
        kt_host, bias_bcast = _prep_weights(weight_1, weight_2, bias)
        wmap = {"kt": kt_host, "bias_bcast": bias_bcast}
    else:
        assert mode.startswith("bd_"), mode
        w1bd, w2bd, bias_bcast = _prep_weights_bd(
            weight_1, weight_2, bias, mode[3:])
        wmap = {"w1bd": w1bd, "w2bd": w2bd, "bias_bcast": bias_bcast}
    return [
        {"x": np.ascontiguousarray(xf[i * n_tok:(i + 1) * n_tok]), **wmap}
        for i in range(N_CORES)
    ]


def kernel(x, weight_1, weight_2, bias, _n_tok=TOK_PER_CORE, _mode="bd_f32r",
           _reps=1, _trace=False):
    """_mode: "bd_f32r" | "bd_f32" | "bd_bf16" (block-diag factored) or
    "dense" / "dense_f32r" (dense-K fallback)."""
    global LAST_RESULT
    from concourse import bass_utils

    in_maps = prep_in_maps(x, weight_1, weight_2, bias, _n_tok, _mode)
    nc = build_for_mode(_n_tok, _mode, _reps)
    res = bass_utils.run_bass_kernel_spmd(
        nc, in_maps, core_ids=list(range(N_CORES)), trace=bool(_trace))
    LAST_RESULT = res
    out = np.concatenate(
        [np.asarray(res.results[i]["y"], dtype=np.float32)
         for i in range(N_CORES)], axis=0)
    if _mode == "v2":
        out += np.asarray(bias, dtype=np.float32)
    if _n_tok == TOK_PER_CORE:
        out = out.reshape(B, S, OUT)
    return out

